# revision 8
# baseline (speedup 1.0000x reference)
"""Binary 3x3 conv (sign(x) * sign(w) conv, scaled by alpha) on 8 TRN2 NeuronCores.

Strategy
--------
- Data-parallel over batch: 32 images -> 4 per core; weights replicated.
- Conv lowered to 9 shifted matmuls accumulating in PSUM, contracting over
  input channels (C=256) placed on SBUF partitions (2 chunks of 128).
- Binarization is exact: sign values ±1/0 are exact in fp8e4m3, products are
  ±1/0, PSUM accumulates in fp32, sums ≤ 2304 are exact integers -> the
  result is bit-identical to the f32 reference.
- fp8 DoubleRow perf mode packs both 128-channel chunks into one matmul
  (effective K=256, 2 MACs/cell/cycle) -> 504 matmuls/core at ~194ns issue
  rate = ~98us PE floor (the fp8 roofline for direct conv).
- Activation planes stored with a single pad column per row (57 wide): a
  row's right halo IS the next row's left pad, so every 3x3 tap window is a
  *contiguous* 1-D span of the flattened plane. One garbage output column
  per row (c=0), dropped during PSUM->SBUF eviction.
- Latency hiding: output tiles computed t-outer/s-inner so early tiles only
  need early input rows; x is loaded in 14-row chunks alternating between
  the two channel chunks and binarized (ScalarE Sign) as chunks land;
  weight sign runs on VectorE (min(w*2^100,1) then max(.,-1) — exact) so it
  never contends with the activation signs; dummy matmuls on a zero scratch
  tile keep the PE HAM clock gate warm through the prologue.
"""

import numpy as np

import concourse.bacc as bacc
import concourse.bass as bass
import concourse.mybir as mybir
from concourse import tile
from concourse.bass_utils import run_bass_kernel_spmd

N_CORES = 8
B, C, H, W = 32, 256, 56, 56
BP = B // N_CORES  # images per core
O = 256
PW = W + 1  # padded row width: one shared pad column per row
PLANE = 3312  # fp8 elems per (img, cc) plane; 58*57=3306 used, %16==0
GUARD = 16  # header so the (dy=-1,dx=-1) tap of cc0 stays in-bounds
PAD_FREE = GUARD + 2 * PLANE

ROWS_PER_TILE = 8
NT = H // ROWS_PER_TILE  # 7 pixel tiles per image
FD = ROWS_PER_TILE * PW  # 456 matmul free dim (<=512: one PSUM bank)

XCH = 4  # DMA/sign chunks per (img, cc) plane
CH_ROWS = H // XCH  # 14
N_WARMUP_MM = 30  # dummy matmuls bridging the prologue

BIG = float(2.0**100)

F8 = mybir.dt.float8e4
F32 = mybir.dt.float32

_compiled = None


def _build():
    nc = bacc.Bacc("TRN2", target_bir_lowering=False, debug=False, num_devices=N_CORES)

    x_dram = nc.dram_tensor("x", [BP, C, H, W], F32, kind="ExternalInput")
    wt_dram = nc.dram_tensor("wt", [C, 9, O], F32, kind="ExternalInput")
    alpha_dram = nc.dram_tensor("alpha", [1], F32, kind="ExternalInput")
    out_dram = nc.dram_tensor("out", [BP, O, H, W], F32, kind="ExternalOutput")

    with tile.TileContext(nc) as tc:
        with (
            tc.tile_pool(name="const", bufs=1) as const_pool,
            tc.tile_pool(name="xin", bufs=8) as xin_pool,
            tc.tile_pool(name="wstage", bufs=3) as wstage_pool,
            tc.tile_pool(name="oplane", bufs=3) as out_pool,
            tc.tile_pool(name="psum", bufs=7, space=bass.MemorySpace.PSUM) as psum_pool,
            tc.tile_pool(name="wpsum", bufs=1, space=bass.MemorySpace.PSUM) as wpsum_pool,
        ):
            # --- PE warm-up: matmuls on a zeroed scratch tile, no data deps
            # (pair stride must be 16-aligned: pad the scratch to 464 wide)
            warm = const_pool.tile([128, 2, 464], F8, name="warm")
            nc.gpsimd.memset(warm[:], 0)
            wps = wpsum_pool.tile([128, FD], F32, name="wps")
            for _ in range(N_WARMUP_MM):
                nc.tensor.matmul(
                    wps[:],
                    warm[:, :, 0:128],
                    warm[:, :, 0:FD],
                    start=True,
                    stop=True,
                    perf_mode=mybir.MatmulPerfMode.DoubleRow,
                )

            # alpha broadcast to all 128 partitions (scalar-engine DMA ring)
            alpha_sb = const_pool.tile([128, 1], F32, name="alpha_sb")
            nc.scalar.dma_start(alpha_sb[:], alpha_dram.ap().partition_broadcast(128))

            # per-tap weight tiles: [c_part, cc, o] f32, signed on VectorE.
            # wt HBM layout is [c, s, o]: c stride 9*O, cc stride 128*9*O.
            w8s = [const_pool.tile([128, 2, O], F8, name=f"w8_{s}") for s in range(9)]
            for s in range(9):
                wstage = wstage_pool.tile([128, 2, O], F32, name="wstage", tag="ws")
                wtmp = wstage_pool.tile([128, 2, O], F32, name="wtmp", tag="wt")
                src = bass.AP(wt_dram, s * O, [[9 * O, 128], [128 * 9 * O, 2], [1, O]])
                nc.sync.dma_start(wstage[:], src)
                nc.vector.tensor_scalar(
                    wtmp[:], wstage[:], BIG, 1.0,
                    op0=mybir.AluOpType.mult, op1=mybir.AluOpType.min,
                )
                nc.vector.tensor_scalar(
                    w8s[s][:], wtmp[:], -1.0, None, op0=mybir.AluOpType.max
                )

            # per-image padded fp8 activation planes (both cc chunks in one
            # tile: the DoubleRow rhs AP needs a fixed stride between chunks)
            pads = [
                const_pool.tile([128, PAD_FREE], F8, name=f"pad{img}")
                for img in range(BP)
            ]
            for img in range(BP):
                ph, pstep = pads[img][:].tensor, pads[img][:].ap[0][0]
                for cc in range(2):
                    base = GUARD + cc * PLANE
                    # top pad row (+ leading guard elem); bottom pad row
                    # (+ the sliver the widest tap reads); left pad column
                    nc.gpsimd.memset(
                        bass.AP(ph, base - 1, [[pstep, 128], [1, PW + 1]]), 0
                    )
                    nc.gpsimd.memset(
                        bass.AP(ph, base + 57 * PW, [[pstep, 128], [1, PLANE - 57 * PW]]),
                        0,
                    )
                    nc.gpsimd.memset(
                        bass.AP(ph, base + PW, [[pstep, 128], [PW, H], [1, 1]]), 0
                    )

            # x loads: 14-row chunks, alternating cc so both halves of a row
            # band arrive together (matmul t-tiles consume rows in order)
            for img in range(BP):
                ph, pstep = pads[img][:].tensor, pads[img][:].ap[0][0]
                for ch in range(XCH):
                    h0 = ch * CH_ROWS
                    for cc in range(2):
                        xin = xin_pool.tile([128, CH_ROWS, W], F32, name="xin", tag="xi")
                        nc.sync.dma_start(
                            xin[:],
                            x_dram[img, cc * 128 : (cc + 1) * 128, h0 : h0 + CH_ROWS],
                        )
                        dst = bass.AP(
                            ph,
                            GUARD + cc * PLANE + (h0 + 1) * PW + 1,
                            [[pstep, 128], [PW, CH_ROWS], [1, W]],
                        )
                        nc.scalar.sign(dst, xin[:])

            # conv: per output tile, 9 shifted fp8 DoubleRow matmuls then a
            # VectorE eviction (drop garbage column, scale by alpha)
            for img in range(BP):
                ph, pstep = pads[img][:].tensor, pads[img][:].ap[0][0]
                for oc in range(2):
                    oplane = out_pool.tile([128, H, W], F32, name="oplane")
                    for t in range(NT):
                        ps = psum_pool.tile([128, FD], F32, name="ps", tag="ps")
                        for s in range(9):
                            dy, dx = s // 3 - 1, s % 3 - 1
                            wts = w8s[s][:]
                            lhsT = bass.AP(
                                wts.tensor,
                                oc * 128,
                                [[wts.ap[0][0], 128], [O, 2], [1, 128]],
                            )
                            rhs = bass.AP(
                                ph,
                                GUARD + (ROWS_PER_TILE * t + 1 + dy) * PW + dx,
                                [[pstep, 128], [PLANE, 2], [1, FD]],
                            )
                            nc.tensor.matmul(
                                ps[:],
                                lhsT,
                                rhs,
                                start=(s == 0),
                                stop=(s == 8),
                                perf_mode=mybir.MatmulPerfMode.DoubleRow,
                            )
                        pb = ps[:]
                        src = bass.AP(
                            pb.tensor,
                            pb.offset + 1,
                            [[pb.ap[0][0], 128], [PW, ROWS_PER_TILE], [1, W]],
                        )
                        dst = oplane[:, ROWS_PER_TILE * t : ROWS_PER_TILE * (t + 1), :]
                        nc.vector.tensor_scalar_mul(dst, src, alpha_sb[:, 0:1])
                    # split the store so it starts before the last eviction
                    half = (NT // 2) * ROWS_PER_TILE  # rows 0..23 / 24..55
                    och = out_dram[img, oc * 128 : (oc + 1) * 128]
                    nc.sync.dma_start(och[:, :half, :], oplane[:, :half, :])
                    nc.sync.dma_start(och[:, half:, :], oplane[:, half:, :])

    nc.compile()
    return nc


def _get_compiled():
    global _compiled
    if _compiled is None:
        _compiled = _build()
    return _compiled


def run(x: np.ndarray, weight: np.ndarray, alpha: np.ndarray, **kw):
    nc = _get_compiled()
    # [o,c,ky,kx] -> [c, ky*3+kx, o] so channels land on partitions directly
    wt = np.ascontiguousarray(weight.transpose(1, 2, 3, 0).reshape(C, 9, O)).astype(
        np.float32
    )
    x = np.ascontiguousarray(x, dtype=np.float32)
    alpha = np.ascontiguousarray(alpha, dtype=np.float32)
    in_maps = [
        {"x": x[i * BP : (i + 1) * BP], "wt": wt, "alpha": alpha}
        for i in range(N_CORES)
    ]
    res = run_bass_kernel_spmd(nc, in_maps, list(range(N_CORES)), **kw)
    return np.concatenate([r["out"] for r in res.results], axis=0), res


def kernel(x: np.ndarray, weight: np.ndarray, alpha: np.ndarray) -> np.ndarray:
    return run(x, weight, alpha)[0]


# revision 12
# speedup vs baseline: 1.0752x; 1.0752x over previous
"""Binary 3x3 conv (sign(x) * sign(w) conv, scaled by alpha) on 8 TRN2 NeuronCores.

Strategy
--------
- Data-parallel over batch: 32 images -> 4 per core; weights replicated.
- Conv lowered to 9 shifted matmuls accumulating in PSUM, contracting over
  input channels (C=256) placed on SBUF partitions (2 chunks of 128).
- Binarization is exact: sign values ±1/0 are exact in fp8e4m3, products are
  ±1/0, PSUM accumulates in fp32, sums ≤ 2304 are exact integers -> the
  result is bit-identical to the f32 reference.
- fp8 DoubleRow perf mode packs both 128-channel chunks into one matmul
  (effective K=256, 2 MACs/cell/cycle) -> 504 matmuls/core at ~194ns issue
  rate = ~98us PE floor (the fp8 roofline for direct conv).
- Activation planes stored with a single pad column per row (57 wide): a
  row's right halo IS the next row's left pad, so every 3x3 tap window is a
  *contiguous* 1-D span of the flattened plane. One garbage output column
  per row (c=0), dropped during PSUM->SBUF eviction.
- Latency hiding: output tiles computed t-outer/s-inner so early tiles only
  need early input rows; x is loaded in 14-row chunks alternating between
  the two channel chunks and binarized (ScalarE Sign) as chunks land;
  weight sign runs on VectorE (min(w*2^100,1) then max(.,-1) — exact) so it
  never contends with the activation signs; dummy matmuls on a zero scratch
  tile keep the PE HAM clock gate warm through the prologue.
"""

import numpy as np

import concourse.bacc as bacc
import concourse.bass as bass
import concourse.mybir as mybir
from concourse import tile
from concourse.bass_utils import run_bass_kernel_spmd

N_CORES = 8
B, C, H, W = 32, 256, 56, 56
BP = B // N_CORES  # images per core
O = 256
PW = W + 1  # padded row width: one shared pad column per row
PLANE = 3312  # fp8 elems per (img, cc) plane; 58*57=3306 used, %16==0
GUARD = 16  # header so the (dy=-1,dx=-1) tap of cc0 stays in-bounds
PAD_FREE = GUARD + 2 * PLANE

ROWS_PER_TILE = 8
NT = H // ROWS_PER_TILE  # 7 pixel tiles per image
FD = ROWS_PER_TILE * PW  # 456 matmul free dim (<=512: one PSUM bank)

N_WARMUP_MM = 40  # dummy matmuls bridging the prologue

BIG = float(2.0**100)

F8 = mybir.dt.float8e4
F32 = mybir.dt.float32

_compiled = None


def _build():
    nc = bacc.Bacc("TRN2", target_bir_lowering=False, debug=False, num_devices=N_CORES)

    x_dram = nc.dram_tensor("x", [BP, C, H, W], F32, kind="ExternalInput")
    wt_dram = nc.dram_tensor("wt", [C, 9, O], F32, kind="ExternalInput")
    alpha_dram = nc.dram_tensor("alpha", [1], F32, kind="ExternalInput")
    out_dram = nc.dram_tensor("out", [BP, O, H, W], F32, kind="ExternalOutput")

    with tile.TileContext(nc) as tc:
        with (
            tc.tile_pool(name="const", bufs=1) as const_pool,
            tc.tile_pool(name="xin", bufs=8) as xin_pool,
            tc.tile_pool(name="wstage", bufs=3) as wstage_pool,
            tc.tile_pool(name="oplane", bufs=3) as out_pool,
            tc.tile_pool(name="psum", bufs=7, space=bass.MemorySpace.PSUM) as psum_pool,
            tc.tile_pool(name="wpsum", bufs=1, space=bass.MemorySpace.PSUM) as wpsum_pool,
        ):
            # --- PE warm-up: matmuls on a zeroed scratch tile, no data deps
            # (pair stride must be 16-aligned: pad the scratch to 464 wide)
            warm = const_pool.tile([128, 2, 464], F8, name="warm")
            nc.gpsimd.memset(warm[:], 0)
            wps = wpsum_pool.tile([128, FD], F32, name="wps")
            for _ in range(N_WARMUP_MM):
                nc.tensor.matmul(
                    wps[:],
                    warm[:, :, 0:128],
                    warm[:, :, 0:FD],
                    start=True,
                    stop=True,
                    perf_mode=mybir.MatmulPerfMode.DoubleRow,
                )

            # alpha broadcast to all 128 partitions (scalar-engine DMA ring)
            alpha_sb = const_pool.tile([128, 1], F32, name="alpha_sb")
            nc.scalar.dma_start(alpha_sb[:], alpha_dram.ap().partition_broadcast(128))

            # per-tap weight tiles: [c_part, cc, o] f32, signed on VectorE
            # (min(w*2^100, 1) then max(., -1) — exact ±1/0, keeps ScalarE
            # free for the activation signs).
            # wt HBM layout is [c, s, o]: c stride 9*O, cc stride 128*9*O.
            w8s = [const_pool.tile([128, 2, O], F8, name=f"w8_{s}") for s in range(9)]

            def load_tap_weights(s):
                wstage = wstage_pool.tile([128, 2, O], F32, name="wstage", tag="ws")
                wtmp = wstage_pool.tile([128, 2, O], F32, name="wtmp", tag="wt")
                src = bass.AP(wt_dram, s * O, [[9 * O, 128], [128 * 9 * O, 2], [1, O]])
                nc.sync.dma_start(wstage[:], src)
                nc.vector.tensor_scalar(
                    wtmp[:], wstage[:], BIG, 1.0,
                    op0=mybir.AluOpType.mult, op1=mybir.AluOpType.min,
                )
                nc.vector.tensor_scalar(
                    w8s[s][:], wtmp[:], -1.0, None, op0=mybir.AluOpType.max
                )

            # per-image padded fp8 activation planes (both cc chunks in one
            # tile: the DoubleRow rhs AP needs a fixed stride between chunks)
            pads = [
                const_pool.tile([128, PAD_FREE], F8, name=f"pad{img}")
                for img in range(BP)
            ]
            for img in range(BP):
                ph, pstep = pads[img][:].tensor, pads[img][:].ap[0][0]
                for cc in range(2):
                    base = GUARD + cc * PLANE
                    # top pad row (+ leading guard elem); bottom pad row
                    # (+ the sliver the widest tap reads); left pad column
                    nc.gpsimd.memset(
                        bass.AP(ph, base - 1, [[pstep, 128], [1, PW + 1]]), 0
                    )
                    nc.gpsimd.memset(
                        bass.AP(ph, base + 57 * PW, [[pstep, 128], [1, PLANE - 57 * PW]]),
                        0,
                    )
                    nc.gpsimd.memset(
                        bass.AP(ph, base + PW, [[pstep, 128], [PW, H], [1, 1]]), 0
                    )

            # x loads (sync ring is FIFO: order = priority). A matmul's rhs AP
            # spans all of the cc0 plane (pair-dim stride), so img0 cc0 loads
            # first in big chunks; cc1 in small row chunks so the s=0 t-tiles
            # unlock progressively; taps 1-3 squeezed between so s=1..3
            # weights are ready when the matmul stream reaches them.
            def load_chunk(img, cc, h0, rows):
                ph, pstep = pads[img][:].tensor, pads[img][:].ap[0][0]
                xin = xin_pool.tile([128, rows, W], F32, name="xin", tag="xi")
                nc.sync.dma_start(
                    xin[:], x_dram[img, cc * 128 : (cc + 1) * 128, h0 : h0 + rows]
                )
                dst = bass.AP(
                    ph,
                    GUARD + cc * PLANE + (h0 + 1) * PW + 1,
                    [[pstep, 128], [PW, rows], [1, W]],
                )
                nc.scalar.sign(dst, xin[:])

            load_tap_weights(0)
            load_chunk(0, 0, 0, 28)
            load_chunk(0, 0, 28, 28)
            for s in (1, 2, 3):
                load_tap_weights(s)
            for ch in range(4):
                load_chunk(0, 1, ch * 14, 14)
            for s in (4, 5, 6, 7, 8):
                load_tap_weights(s)
            for img in range(1, BP):
                for cc in range(2):
                    load_chunk(img, cc, 0, 28)
                    load_chunk(img, cc, 28, 28)

            # conv: 9 shifted fp8 DoubleRow matmuls per output tile, s-outer /
            # t-inner (one tap across all 7 PSUM banks before the next tap),
            # then VectorE evictions (drop garbage column, scale by alpha)
            for img in range(BP):
                ph, pstep = pads[img][:].tensor, pads[img][:].ap[0][0]
                for oc in range(2):
                    psums = [
                        psum_pool.tile([128, FD], F32, name="ps", tag="ps")
                        for _ in range(NT)
                    ]
                    for s in range(9):
                        dy, dx = s // 3 - 1, s % 3 - 1
                        wts = w8s[s][:]
                        lhsT = bass.AP(
                            wts.tensor,
                            oc * 128,
                            [[wts.ap[0][0], 128], [O, 2], [1, 128]],
                        )
                        for t in range(NT):
                            rhs = bass.AP(
                                ph,
                                GUARD + (ROWS_PER_TILE * t + 1 + dy) * PW + dx,
                                [[pstep, 128], [PLANE, 2], [1, FD]],
                            )
                            nc.tensor.matmul(
                                psums[t][:],
                                lhsT,
                                rhs,
                                start=(s == 0),
                                stop=(s == 8),
                                perf_mode=mybir.MatmulPerfMode.DoubleRow,
                            )
                    oplane = out_pool.tile([128, H, W], F32, name="oplane")
                    for t in range(NT):
                        pb = psums[t][:]
                        src = bass.AP(
                            pb.tensor,
                            pb.offset + 1,
                            [[pb.ap[0][0], 128], [PW, ROWS_PER_TILE], [1, W]],
                        )
                        dst = oplane[:, ROWS_PER_TILE * t : ROWS_PER_TILE * (t + 1), :]
                        nc.vector.tensor_scalar_mul(dst, src, alpha_sb[:, 0:1])
                    # split the store so it starts before the last eviction
                    half = (NT // 2) * ROWS_PER_TILE  # rows 0..23 / 24..55
                    och = out_dram[img, oc * 128 : (oc + 1) * 128]
                    nc.sync.dma_start(och[:, :half, :], oplane[:, :half, :])
                    nc.sync.dma_start(och[:, half:, :], oplane[:, half:, :])

    nc.compile()
    return nc


def _get_compiled():
    global _compiled
    if _compiled is None:
        _compiled = _build()
    return _compiled


def run(x: np.ndarray, weight: np.ndarray, alpha: np.ndarray, **kw):
    nc = _get_compiled()
    # [o,c,ky,kx] -> [c, ky*3+kx, o] so channels land on partitions directly
    wt = np.ascontiguousarray(weight.transpose(1, 2, 3, 0).reshape(C, 9, O)).astype(
        np.float32
    )
    x = np.ascontiguousarray(x, dtype=np.float32)
    alpha = np.ascontiguousarray(alpha, dtype=np.float32)
    in_maps = [
        {"x": x[i * BP : (i + 1) * BP], "wt": wt, "alpha": alpha}
        for i in range(N_CORES)
    ]
    res = run_bass_kernel_spmd(nc, in_maps, list(range(N_CORES)), **kw)
    return np.concatenate([r["out"] for r in res.results], axis=0), res


def kernel(x: np.ndarray, weight: np.ndarray, alpha: np.ndarray) -> np.ndarray:
    return run(x, weight, alpha)[0]
